# revision 3
# baseline (speedup 1.0000x reference)
"""MoE gating kernel (logits -> softmax -> top-2 mask) for 8 trn2 NeuronCores.

Math: logits = x @ W.T + b  [B,S,E]; weights = softmax(logits, -1);
gated = weights masked to per-token top-2.  Returns (gated.T, weights.T),
both [E, B, S] fp32.

Strategy:
  - Shard tokens (B*S = 65536) across 8 cores, 8192 tokens each.
  - fp32 precision via scaled fp16 splits (PE fp16 matmul is 4x faster
    than fp32 and 2-byte dtype enables DMA xbar transpose):
        x ~= A + 2^-11 * B      (A, B fp16)
        W ~= 2^-8*C + 2^-19*D   (C, D fp16)
        logits*2^8 = A@C.T + 2^-11*(A@D.T + B@C.T)   (+ 2^8*b)
    dropped cross term B@D * 2^-30 is ~1e-7 of logit scale.
  - Per core, per 1024-token group: DMA-transpose A,B blocks from DRAM
    straight into SBUF as [128 d, 1024 t] (host pre-blocks so each source
    is a contiguous [1024, 128] fp16 block), 3 matmuls per 128-d chunk
    accumulating logits.T [16, 512] in PSUM, combine scales on DVE,
    PE-transpose back to [128 t, 16 e], then per 128-token tile:
    max8 -> 2nd-max threshold, exp(scale=2^-8) with fused row-sum,
    reciprocal, w = E*recip, gated = (logits >= thr) * w.
  - Outputs accumulate in SBUF in [(tile,e), t] layout via PE transpose
    and are written once at the end with a strided DMA.
"""

import functools

import numpy as np

NUM_CORES = 8
TOK_PER_CORE = 8192
GROUPS = 8  # groups per core
GTOK = 1024  # tokens per group
TILES = 8  # 128-token tiles per group
CHUNKS = 8  # d chunks of 128
D = 1024
E = 16

# scale exponents for the fp16 splits
XS = 11  # x = A + 2^-XS * B
WS = 8  # W = 2^-WS * C + ...
WS2 = 19  # ... + 2^-WS2 * D

TRACE = False  # set by test.py to capture an NTFF profile
LAST_RESULTS = None  # BassKernelResults stash for test.py


@functools.lru_cache(maxsize=2)
def _build(has_b: bool):
    from concourse import bacc, mybir
    import concourse.bass as bass
    import concourse.tile as tile
    from concourse.masks import make_identity

    f16 = mybir.dt.float16
    f32 = mybir.dt.float32
    Exp = mybir.ActivationFunctionType.Exp
    Op = mybir.AluOpType

    nc = bacc.Bacc(
        "TRN2", target_bir_lowering=False, debug=False, num_devices=NUM_CORES
    )

    a_dram = nc.dram_tensor("a_t", [GROUPS, CHUNKS, GTOK, 128], f16, kind="ExternalInput").ap()
    b_dram = nc.dram_tensor("b_t", [GROUPS, CHUNKS, GTOK, 128], f16, kind="ExternalInput").ap()
    ct_dram = nc.dram_tensor("ct", [128, CHUNKS, E], f16, kind="ExternalInput").ap()
    dt_dram = nc.dram_tensor("dt", [128, CHUNKS, E], f16, kind="ExternalInput").ap()
    if has_b:
        bcd_dram = nc.dram_tensor("bcd", [1, 2 * E], f16, kind="ExternalInput").ap()
    wts_dram = nc.dram_tensor("wts", [E, TOK_PER_CORE], f32, kind="ExternalOutput")
    gated_dram = nc.dram_tensor("gated", [E, TOK_PER_CORE], f32, kind="ExternalOutput")

    with tile.TileContext(nc) as tc:
        with (
            tc.tile_pool(name="consts", bufs=1) as consts,
            tc.tile_pool(name="xt", bufs=2) as xt_pool,
            tc.tile_pool(name="lg", bufs=2) as lg_pool,
            tc.tile_pool(name="sm", bufs=2) as sm_pool,
            tc.tile_pool(name="oacc", bufs=1) as oacc_pool,
            tc.tile_pool(name="ps1", bufs=2, space="PSUM") as ps1_pool,
            tc.tile_pool(name="ps2", bufs=2, space="PSUM") as ps2_pool,
            tc.tile_pool(name="pslgt", bufs=2, space="PSUM") as pslgt_pool,
            tc.tile_pool(name="psout", bufs=2, space="PSUM") as psout_pool,
        ):
            ct_sb = consts.tile([128, CHUNKS, E], f16)
            dt_sb = consts.tile([128, CHUNKS, E], f16)
            nc.sync.dma_start(out=ct_sb, in_=ct_dram)
            nc.sync.dma_start(out=dt_sb, in_=dt_dram)
            ident = consts.tile([128, 128], f32)
            make_identity(nc, ident)
            if has_b:
                bcd_sb = consts.tile([1, 2 * E], f16)
                nc.sync.dma_start(out=bcd_sb, in_=bcd_dram)
                ones_sb = consts.tile([1, 512], f16)
                nc.vector.memset(ones_sb, 1.0)

            # output accumulators: partition = (tile, e), free = (group, t)
            w_acc = oacc_pool.tile([128, GROUPS, 128], f32)
            g_acc = oacc_pool.tile([128, GROUPS, 128], f32)

            for g in range(GROUPS):
                aT = xt_pool.tile([128, CHUNKS, GTOK], f16, tag="aT")
                bT = xt_pool.tile([128, CHUNKS, GTOK], f16, tag="bT")
                for k in range(CHUNKS):
                    nc.sync.dma_start_transpose(out=aT[:, k, :], in_=a_dram[g, k])
                    nc.sync.dma_start_transpose(out=bT[:, k, :], in_=b_dram[g, k])

                # scaled logits.T for the group: lg = 2^WS * logits
                lg = lg_pool.tile([E, GTOK], f32)
                for half in range(2):
                    n0 = half * 512
                    s1 = ps1_pool.tile([E, 512], f32)
                    s2 = ps2_pool.tile([E, 512], f32)
                    for k in range(CHUNKS):
                        last = (k == CHUNKS - 1) and not has_b
                        nc.tensor.matmul(
                            s1, lhsT=ct_sb[:, k, :], rhs=aT[:, k, n0 : n0 + 512],
                            start=(k == 0), stop=last,
                        )
                        nc.tensor.matmul(
                            s2, lhsT=dt_sb[:, k, :], rhs=aT[:, k, n0 : n0 + 512],
                            start=(k == 0), stop=False,
                        )
                        nc.tensor.matmul(
                            s2, lhsT=ct_sb[:, k, :], rhs=bT[:, k, n0 : n0 + 512],
                            start=False, stop=last,
                        )
                    if has_b:
                        nc.tensor.matmul(
                            s1, lhsT=bcd_sb[:, 0:E], rhs=ones_sb,
                            start=False, stop=True,
                        )
                        nc.tensor.matmul(
                            s2, lhsT=bcd_sb[:, E : 2 * E], rhs=ones_sb,
                            start=False, stop=True,
                        )
                    # lg = s1 + 2^-XS * s2  (one PSUM input per instruction)
                    nc.scalar.mul(lg[:, n0 : n0 + 512], s2, float(2.0**-XS))
                    nc.vector.tensor_add(
                        lg[:, n0 : n0 + 512], lg[:, n0 : n0 + 512], s1
                    )

                # transpose back to [128 t, E] per 128-token tile
                lgt_ps = pslgt_pool.tile([128, TILES, E], f32)
                for i in range(TILES):
                    nc.tensor.transpose(
                        lgt_ps[:, i, :], lg[:, 128 * i : 128 * (i + 1)], ident[:E, :E]
                    )
                lgt = sm_pool.tile([128, TILES, E], f32, tag="lgt")
                nc.vector.tensor_copy(lgt, lgt_ps)

                w_grp = sm_pool.tile([128, TILES, E], f32, tag="wg")
                g_grp = sm_pool.tile([128, TILES, E], f32, tag="gg")
                for i in range(TILES):
                    lgt_i = lgt[:, i, :]
                    m8 = sm_pool.tile([128, 8], f32, tag="m8")
                    nc.vector.max(m8, lgt_i)
                    ex = sm_pool.tile([128, E], f32, tag="ex")
                    ssum = sm_pool.tile([128, 1], f32, tag="ssum")
                    nc.scalar.activation(
                        ex, lgt_i, func=Exp, scale=float(2.0**-WS), accum_out=ssum
                    )
                    rec = sm_pool.tile([128, 1], f32, tag="rec")
                    nc.vector.reciprocal(rec, ssum)
                    nc.vector.tensor_scalar_mul(w_grp[:, i, :], ex, rec)
                    # gated = (logits >= 2nd max) * w
                    nc.vector.scalar_tensor_tensor(
                        out=g_grp[:, i, :],
                        in0=lgt_i, scalar=m8[:, 1:2], in1=w_grp[:, i, :],
                        op0=Op.is_ge, op1=Op.mult,
                    )

                # transpose outputs to [(tile,e), t] and stash in accumulators
                ps_o = psout_pool.tile([128, 256], f32)
                nc.tensor.transpose(ps_o[:, 0:128], w_grp, ident)
                nc.tensor.transpose(ps_o[:, 128:256], g_grp, ident)
                nc.scalar.copy(w_acc[:, g, :], ps_o[:, 0:128])
                nc.scalar.copy(g_acc[:, g, :], ps_o[:, 128:256])

            # final writeback: partition p=(tile,e) row holds [group, t];
            # dram addr = e*8192 + g*1024 + tile*128 + t
            out_ap = [[128, TILES], [TOK_PER_CORE, E], [GTOK, GROUPS], [1, 128]]
            nc.sync.dma_start(
                out=bass.AP(tensor=wts_dram, offset=0, ap=list(out_ap)), in_=w_acc
            )
            nc.sync.dma_start(
                out=bass.AP(tensor=gated_dram, offset=0, ap=list(out_ap)), in_=g_acc
            )

    nc.compile()
    return nc


def _split_x(xf):
    a = xf.astype(np.float16)
    b = ((xf - a.astype(np.float32)) * np.float32(2.0**XS)).astype(np.float16)
    return a, b


def kernel(x, W, b):
    global LAST_RESULTS
    from concourse.bass_utils import run_bass_kernel_spmd

    x = np.ascontiguousarray(np.asarray(x, dtype=np.float32))
    W = np.ascontiguousarray(np.asarray(W, dtype=np.float32))
    b = np.ascontiguousarray(np.asarray(b, dtype=np.float32))
    Bb, S, Dd = x.shape
    ntok = Bb * S
    assert (ntok, Dd) == (NUM_CORES * TOK_PER_CORE, D) and W.shape == (E, D)

    xf = x.reshape(ntok, D)
    A, Bx = _split_x(xf)
    # per-core blocked layout [groups, chunks, 1024 t, 128 d], contiguous blocks
    def blocked(arr, c):
        sh = arr[c * TOK_PER_CORE : (c + 1) * TOK_PER_CORE]
        return np.ascontiguousarray(
            sh.reshape(GROUPS, GTOK, CHUNKS, 128).transpose(0, 2, 1, 3)
        )

    C = (W * np.float32(2.0**WS)).astype(np.float16)
    Dw = ((W - C.astype(np.float32) * np.float32(2.0**-WS)) * np.float32(2.0**WS2)).astype(np.float16)
    # [128 d_lo, chunks, E]: ct[p, k, e] = C[e, k*128+p]
    ct = np.ascontiguousarray(C.T.reshape(CHUNKS, 128, E).transpose(1, 0, 2))
    dt = np.ascontiguousarray(Dw.T.reshape(CHUNKS, 128, E).transpose(1, 0, 2))

    has_b = bool(np.any(b))
    in_maps = []
    for c in range(NUM_CORES):
        m = {"a_t": blocked(A, c), "b_t": blocked(Bx, c), "ct": ct, "dt": dt}
        if has_b:
            bc = (b * np.float32(2.0**WS)).astype(np.float16)
            bd = ((b - bc.astype(np.float32) * np.float32(2.0**-WS)) * np.float32(2.0**WS2)).astype(np.float16)
            m["bcd"] = np.concatenate([bc, bd]).reshape(1, 2 * E)
        in_maps.append(m)

    nc = _build(has_b)
    res = run_bass_kernel_spmd(
        nc, in_maps, core_ids=list(range(NUM_CORES)), trace=TRACE
    )
    LAST_RESULTS = res

    wts = np.concatenate([r["wts"] for r in res.results], axis=1)
    gated = np.concatenate([r["gated"] for r in res.results], axis=1)
    return (
        gated.reshape(E, Bb, S).astype(np.float32),
        wts.reshape(E, Bb, S).astype(np.float32),
    )
